# revision 34
# baseline (speedup 1.0000x reference)
"""DN4++ retrieval kernel for 8 Trainium2 NeuronCores.

Strategy
--------
Math: with eval-mode BN folded into the linear layers, the backbone is
    a1 = relu(x @ W1 + b1); a2 = relu(a1 @ W2f + b2f); a3 = relu(a2 @ W3f + b3f)
    d  = s3*a3 + t3                      (bn3 affine)
    att = sigmoid(a3 @ Wa1f + ba1f -> relu -> @ Wa2 + ba2)
    w   = d * att
The attention factor cancels inside the cosine normalization (att > 0), so
    qn = d_q/||d_q||, sn = d_s/||d_s||, and att is only needed for the
support-value rows ws = d_s*att_s.  Softmax skips max-subtraction
(logits in [-10,10]) and the denominator is obtained from a ones-column in
the feat matmul; the division is deferred through the predictor's relu
(positive homogeneity).

Sharding: support MLP is sharded (1024 rows/core); the tiny descriptors
(snT pre-scaled by 10/||d_s||, and ws rows) are AllGathered; queries are
data-parallel (512/core).  Everything runs in one SPMD launch.
"""

import numpy as np

NCORES = 8
B, S, DIN, H, H2, H4 = 4096, 8192, 512, 128, 64, 32
BQ = B // NCORES          # 512 queries per core
SL = S // NCORES          # 1024 support rows per core
NBQ = BQ // 128           # 4 query blocks / core
NBS = SL // 128           # 8 support blocks / core
NBLK = S // 128           # 64 global support blocks
KC = DIN // 128           # 4 contraction chunks for layer 1
CCA = H2 * SL             # floats of snT (pre-scaled) per rank
CCB = SL * H2             # floats of ws rows per rank
CCN = CCA + CCB           # gathered floats per rank
BN_EPS = 1e-5
TEMP = 10.0
GRP = 3                   # support blocks per exp group (3 psum banks)

_CACHE: dict = {}


def _split_hi_lo(a):
    import ml_dtypes
    a = np.ascontiguousarray(np.asarray(a, np.float32))
    hi = a.astype(ml_dtypes.bfloat16)
    lo = (a - hi.astype(np.float32)).astype(ml_dtypes.bfloat16)
    return hi, lo


def _fold(params):
    g = lambda a: np.asarray(a, np.float64)

    def bn_affine(bn):
        s = g(bn["g"]) / np.sqrt(g(bn["v"]) + BN_EPS)
        t = g(bn["b"]) - g(bn["m"]) * s
        return s, t

    s1, t1 = bn_affine(params["bn1"])
    s2, t2 = bn_affine(params["bn2"])
    s3, t3 = bn_affine(params["bn3"])
    snp, tnp = bn_affine(params["bnp"])

    W1 = g(params["W1"])
    b1 = g(params["b1"])
    W2f = s1[:, None] * g(params["W2"])
    b2f = g(params["b2"]) + t1 @ g(params["W2"])
    W3f = s2[:, None] * g(params["W3"])
    b3f = g(params["b3"]) + t2 @ g(params["W3"])
    Wa1f = s3[:, None] * g(params["Wa1"])
    ba1f = g(params["ba1"]) + t3 @ g(params["Wa1"])
    Wa2 = g(params["Wa2"])
    ba2 = float(np.reshape(g(params["ba2"]), -1)[0])
    Wp1 = g(params["Wp1"])
    bp1 = g(params["bp1"])
    Wp2f = snp[:, None] * g(params["Wp2"])
    bp2f = float(np.reshape(g(params["bp2"]) + tnp @ g(params["Wp2"]), -1)[0])

    f = lambda a: np.ascontiguousarray(np.asarray(a, np.float32))
    vecs = np.zeros((H, 8), np.float32)
    vecs[:, 0] = f(b1)
    vecs[:, 1] = f(b2f)
    vecs[:H2, 2] = f(b3f)
    vecs[:H2, 3] = f(s3)
    vecs[:H2, 4] = f(t3)
    vecs[:H4, 5] = f(ba1f)
    vecs[:, 7] = -ba2            # bias for exp(-z - ba2)

    return dict(
        W1=f(W1), vecs=vecs,
        w2=f(W2f), w3=f(W3f), wa1=f(Wa1f), wa2=f(Wa2),
        wp1=f(Wp1), wp2=f(Wp2f),
        bp1r=f(bp1).reshape(1, H4),
        svec=np.array([[bp2f, 0.0]], np.float32),
    )


def _build(b_q=BQ, s_l=SL, for_sim=False):
    """Build the SPMD Bass program (shape-dependent only)."""
    import concourse.bass as bass
    import concourse.mybir as mybir
    from concourse import masks
    from concourse.tile import TileContext

    dt = mybir.dt
    f32 = dt.float32
    bf16 = dt.bfloat16
    AF = mybir.ActivationFunctionType
    OP = mybir.AluOpType

    nbq = b_q // 128
    nbs = s_l // 128
    nblk = nbs * NCORES
    cca = H2 * s_l
    ccb = s_l * H2
    ccn = cca + ccb
    LN10 = float(np.log(10.0))

    from concourse import bacc

    # Pin Exp/Ln/Square to the one ACT table set containing all three, so the
    # compiler never alternates table loads (~2.7us per switch). Set ids are
    # positional, so preserve dict size/order and only strip functions.
    if not getattr(bacc, "_act_tables_pinned", False):
        import concourse.hw_specs as hw_specs
        _orig_tables = hw_specs.get_activation_tables
        AFT = mybir.ActivationFunctionType
        _strip = {AFT.Exp, AFT.Ln, AFT.Square}
        _target = "natural_log_exp_and_others"

        def _pinned_tables(arch, _orig=_orig_tables):
            full = _orig(arch)
            return {name: (funcs if name == _target else funcs - _strip)
                    for name, funcs in full.items()}

        bacc.get_activation_tables = _pinned_tables
        bacc._act_tables_pinned = True

    if for_sim:
        nc = bacc.Bacc(None, target_bir_lowering=False, debug=True)
    else:
        nc = bacc.Bacc(None)

    # ---------------- I/O ----------------
    # x is pre-transposed on the host: contraction dim on partitions.
    xqT_d = nc.declare_dram_parameter("xqT", [DIN, b_q], f32, isOutput=False)
    xsT_d = nc.declare_dram_parameter("xsT", [DIN, s_l], f32, isOutput=False)
    w1_d = nc.declare_dram_parameter("w1", [DIN, H], f32, isOutput=False)
    w2_d = nc.declare_dram_parameter("w2", [H, H], f32, isOutput=False)
    w3_d = nc.declare_dram_parameter("w3", [H, H2], f32, isOutput=False)
    wa1_d = nc.declare_dram_parameter("wa1", [H2, H4], f32, isOutput=False)
    wa2_d = nc.declare_dram_parameter("wa2", [H4, 1], f32, isOutput=False)
    wp1_d = nc.declare_dram_parameter("wp1", [H2, H4], f32, isOutput=False)
    wp2_d = nc.declare_dram_parameter("wp2", [H4, 1], f32, isOutput=False)
    vecs_d = nc.declare_dram_parameter("vecs", [H, 8], f32, isOutput=False)
    bp1r_d = nc.declare_dram_parameter("bp1r", [1, H4], f32, isOutput=False)
    svec_d = nc.declare_dram_parameter("svec", [1, 2], f32, isOutput=False)
    y_d = nc.declare_dram_parameter("y", [b_q, 1], f32, isOutput=True)

    # internal DRAM
    scr_d = nc.dram_tensor("scr", [s_l], f32)
    cc_in = nc.dram_tensor("cc_in", [ccn], f32)
    cc_out = nc.dram_tensor("cc_out", [NCORES * ccn], f32, addr_space="Shared")

    with TileContext(nc) as tc:
        with (
            tc.tile_pool(name="const", bufs=1) as cpool,
            tc.tile_pool(name="persist", bufs=1) as ppool,
        ):
            # ---------------- constants ----------------
            w1 = cpool.tile([128, KC * H], f32, tag="w1")
            nc.sync.dma_start(
                out=w1[:].rearrange("p (c m) -> p c m", m=H),
                in_=w1_d[:, :].rearrange("(c p) m -> p c m", p=128),
            )
            w2 = cpool.tile([H, H], f32, tag="w2")
            nc.sync.dma_start(out=w2[:], in_=w2_d[:, :])
            w3 = cpool.tile([H, H2], f32, tag="w3")
            nc.sync.dma_start(out=w3[:], in_=w3_d[:, :])
            wa1 = cpool.tile([H2, H4], f32, tag="wa1")
            nc.sync.dma_start(out=wa1[:], in_=wa1_d[:, :])
            wa2 = cpool.tile([H4, 1], f32, tag="wa2")
            nc.sync.dma_start(out=wa2[:], in_=wa2_d[:, :])
            wp1 = cpool.tile([H2, H4], f32, tag="wp1")
            nc.sync.dma_start(out=wp1[:], in_=wp1_d[:, :])
            wp2 = cpool.tile([H4, 1], f32, tag="wp2")
            nc.sync.dma_start(out=wp2[:], in_=wp2_d[:, :])
            vecs = cpool.tile([H, 8], f32, tag="vecs")
            nc.sync.dma_start(out=vecs[:], in_=vecs_d[:, :])
            bp1r = cpool.tile([1, H4], f32, tag="bp1r")
            nc.sync.dma_start(out=bp1r[:], in_=bp1r_d[:, :])
            svec = cpool.tile([1, 2], f32, tag="svec")
            nc.sync.dma_start(out=svec[:], in_=svec_d[:, :])

            id128 = cpool.tile([128, 128], f32, tag="id128")
            masks.make_identity(nc, id128[:])
            ones1 = cpool.tile([1, H2], f32, tag="ones1")
            nc.vector.memset(ones1[:], 1.0)
            v_ln10 = cpool.tile([128, 1], f32, tag="ln10")
            nc.vector.memset(v_ln10[:], LN10)

            v_b1 = vecs[:, 0:1]
            v_b2 = vecs[:, 1:2]
            v_b3 = vecs[0:H2, 2:3]
            v_s3 = vecs[0:H2, 3:4]
            v_t3 = vecs[0:H2, 4:5]
            v_ba1 = vecs[0:H4, 5:6]
            v_nba2 = vecs[:, 7:8]

            # ---------------- persistent tiles ----------------
            dsT = ppool.tile([H2, s_l], f32, tag="dsT")       # raw support desc, T
            dqT = ppool.tile([H2, b_q], f32, tag="dqT")
            snTs = ppool.tile([H2, s_l], f32, tag="snTs")     # 10/||d|| * dsT
            ws_l = ppool.tile([128, nbs * H2], f32, tag="ws_l")
            ss_s = ppool.tile([128, nbs], f32, tag="ss_s")
            ss_q = ppool.tile([128, nbq], f32, tag="ss_q")
            attz_all = ppool.tile([128, nbs], f32, tag="attz_all")
            att_all = ppool.tile([128, nbs], f32, tag="att_all")
            s10r = ppool.tile([128, nbs], f32, tag="s10r")
            qnT = ppool.tile([H2, b_q], f32, tag="qnT")
            qnT2 = ppool.tile([128, b_q], f32, tag="qnT2")
            scale_row = ppool.tile([1, s_l], f32, tag="scale_row")
            # packed layout: even global blocks on partitions 0-63, odd on
            # 64-127; pair p occupies columns p*128:(p+1)*128
            snT_all = ppool.tile([128, nblk * 64], f32, tag="snT_all")
            ws_stage = ppool.tile([128, nbs * H2], f32, tag="ws_stage")
            ws_aug = ppool.tile([128, nblk * (H2 + 1)], bf16, tag="ws_aug")
            sums = ppool.tile([1, b_q], f32, tag="sums")
            featA = ppool.tile([H2, b_q], f32, tag="featA")

            # =============== PHASE 1 ===============
            with (
                tc.tile_pool(name="ph1", bufs=2) as wpool,
                tc.tile_pool(name="ph1x", bufs=4) as xpool,
                tc.tile_pool(name="ps_mlp", bufs=3, space="PSUM") as ps_mlp,
                tc.tile_pool(name="ps_msc", bufs=1, space="PSUM") as ps_msc,
                tc.tile_pool(name="ps_drow", bufs=2, space="PSUM") as ps_drow,
            ):

                def mlp_tile(srcT, r0, R, dT_out):
                    """Backbone for rows [r0, r0+R); writes dT (post-bn3) into
                    dT_out AP ([H2, R]) and returns a3T tile for attention."""
                    h1 = ps_mlp.tile([128, R], f32, tag="mlp")
                    for k in range(KC):
                        xt = xpool.tile([128, R], f32, tag="xt")
                        nc.sync.dma_start(
                            out=xt[:], in_=srcT[k * 128:(k + 1) * 128, r0:r0 + R])
                        wk = w1[:].rearrange("p (c m) -> p c m", m=H)[:, k, :]
                        nc.tensor.matmul(h1[:], wk, xt[:],
                                         start=(k == 0), stop=(k == KC - 1))
                    a1 = wpool.tile([128, R], f32, tag="a1")
                    nc.vector.tensor_scalar(a1[:], h1[:], v_b1, 0.0, OP.add, OP.max)

                    h2 = ps_mlp.tile([128, R], f32, tag="mlp")
                    nc.tensor.matmul(h2[:], w2[:], a1[:], start=True, stop=True)
                    a2 = wpool.tile([128, R], f32, tag="a2")
                    nc.vector.tensor_scalar(a2[:], h2[:], v_b2, 0.0, OP.add, OP.max)

                    h3 = ps_msc.tile([H2, R], f32, tag="misc1")
                    nc.tensor.matmul(h3[:], w3[:], a2[:], start=True, stop=True)
                    a3 = wpool.tile([H2, R], f32, tag="a3")
                    nc.vector.tensor_scalar(a3[:], h3[:], v_b3, 0.0, OP.add, OP.max)

                    # d = s3*a3 + t3   (transposed layout: per-partition affine)
                    nc.vector.tensor_scalar(dT_out, a3[:], v_s3, v_t3, OP.mult, OP.add)
                    return a3

                # ---- support tiles ----
                RT = min(512, s_l)
                for t in range(s_l // RT):
                    dT_out = dsT[:, t * RT:(t + 1) * RT]
                    a3 = mlp_tile(xsT_d, t * RT, RT, dT_out)

                    # attention branch
                    z1 = ps_msc.tile([H4, RT], f32, tag="misc1")
                    nc.tensor.matmul(z1[:], wa1[:], a3[:], start=True, stop=True)
                    za1 = wpool.tile([H4, RT], f32, tag="za1")
                    nc.vector.tensor_scalar(za1[:], z1[:], v_ba1, 0.0, OP.add, OP.max)

                    nb = RT // 128
                    attz = ps_msc.tile([128, nb], f32, tag="misc1")
                    for j in range(nb):
                        nc.tensor.matmul(attz[:, j:j + 1],
                                         za1[:, j * 128:(j + 1) * 128], wa2[:],
                                         start=True, stop=True)
                    nc.vector.tensor_copy(attz_all[:, t * nb:(t + 1) * nb], attz[:])

                    drow = ps_drow.tile([128, nb * H2], f32, tag="drow")
                    scr64 = wpool.tile([128, H2], f32, tag="scr64")
                    for j in range(nb):
                        blk = t * nb + j
                        dblk = drow[:, j * H2:(j + 1) * H2]
                        nc.tensor.transpose(
                            dblk, dT_out[:, j * 128:(j + 1) * 128], id128[0:H2, 0:H2])
                        nc.scalar.activation(scr64[:], dblk, AF.Square,
                                             accum_out=ss_s[:, blk:blk + 1])
                    # stage raw d rows; the att multiply happens after the
                    # (batched) transcendental block below
                    nc.vector.tensor_copy(
                        ws_stage[:, t * nb * H2:(t + 1) * nb * H2], drow[:])

                # batched transcendentals (single ACT table set):
                # scale10r = 10/||d_s|| = exp(-0.5*ln(ss) + ln 10)
                lnss = wpool.tile([128, nbs], f32, tag="lnss")
                nc.scalar.activation(lnss[:], ss_s[:], AF.Ln)
                nc.scalar.activation(s10r[:], lnss[:], AF.Exp, bias=v_ln10[:],
                                     scale=-0.5)
                # att = sigmoid(z + ba2) = 1/(1 + exp(-(z + ba2)))
                enz = wpool.tile([128, nbs], f32, tag="enz")
                nc.scalar.activation(enz[:], attz_all[:], AF.Exp, bias=v_nba2,
                                     scale=-1.0)
                ra = wpool.tile([128, nbs], f32, tag="ra")
                nc.vector.tensor_scalar_add(ra[:], enz[:], 1.0)
                nc.vector.reciprocal(att_all[:], ra[:])
                for b in range(nbs):
                    nc.vector.tensor_scalar_mul(
                        ws_l[:, b * H2:(b + 1) * H2],
                        ws_stage[:, b * H2:(b + 1) * H2], att_all[:, b:b + 1])

                # flatten [128, nbs] -> row vector [1, s_l] with per-block
                # SBUF->SBUF DMAs (address-based, cross partitions)
                for b in range(nbs):
                    nc.sync.dma_start(
                        out=scale_row[:, b * 128:(b + 1) * 128],
                        in_=s10r[:, b:b + 1])

                # snTs = dsT * bcast(scale_row)
                CW = min(512, s_l)
                for c in range(s_l // CW):
                    bc = ps_msc.tile([H2, CW], f32, tag="bq")
                    nc.tensor.matmul(bc[:], ones1[:],
                                     scale_row[:, c * CW:(c + 1) * CW],
                                     start=True, stop=True)
                    nc.vector.tensor_tensor(
                        out=snTs[:, c * CW:(c + 1) * CW],
                        in0=dsT[:, c * CW:(c + 1) * CW], in1=bc[:], op=OP.mult)

                # pack + AllGather
                nc.sync.dma_start(
                    out=cc_in[0:cca].rearrange("(p f) -> p f", p=H2), in_=snTs[:])
                nc.sync.dma_start(
                    out=cc_in[cca:ccn].rearrange("(b p d) -> p b d", p=128, d=H2),
                    in_=ws_l[:].rearrange("p (b d) -> p b d", d=H2))
                nc.gpsimd.collective_compute(
                    "AllGather",
                    mybir.AluOpType.bypass,
                    replica_groups=[list(range(NCORES))],
                    ins=[cc_in[:]],
                    outs=[cc_out[:]],
                )

                # ---- query tile(s) (overlaps the AllGather) ----
                RTQ = min(512, b_q)
                for tq in range(b_q // RTQ):
                    dT_out = dqT[:, tq * RTQ:(tq + 1) * RTQ]
                    mlp_tile(xqT_d, tq * RTQ, RTQ, dT_out)

                    nb = RTQ // 128
                    drow = ps_drow.tile([128, nb * H2], f32, tag="drow")
                    scr64 = wpool.tile([128, H2], f32, tag="scr64")
                    for j in range(nb):
                        blk = tq * nb + j
                        dblk = drow[:, j * H2:(j + 1) * H2]
                        nc.tensor.transpose(
                            dblk, dT_out[:, j * 128:(j + 1) * 128], id128[0:H2, 0:H2])
                        nc.scalar.activation(scr64[:], dblk, AF.Square,
                                             accum_out=ss_q[:, blk:blk + 1])
                    # r_q = exp(-0.5 ln ss)  (no temperature on query side)
                    lnq = wpool.tile([128, nb], f32, tag="lnq")
                    nc.scalar.activation(lnq[:], ss_q[:, tq * nb:(tq + 1) * nb], AF.Ln)
                    rq = wpool.tile([128, nb], f32, tag="rq")
                    nc.scalar.activation(rq[:], lnq[:], AF.Exp, scale=-0.5)
                    for j in range(nb):
                        blk = tq * nb + j
                        dblk = drow[:, j * H2:(j + 1) * H2]
                        qnr = wpool.tile([128, H2], f32, tag="qnr")
                        nc.vector.tensor_scalar_mul(qnr[:], dblk, rq[:, j:j + 1])
                        qb = ps_msc.tile([H2, 128], f32, tag="bq")
                        nc.tensor.transpose(qb[:], qnr[:], id128[:])
                        nc.vector.tensor_copy(qnT[:, blk * 128:(blk + 1) * 128], qb[:])

            # =============== PHASE 2 ===============
            # replicate qnT to partitions 64-127 (for packed sim row-strips)
            nc.sync.dma_start(out=qnT2[0:H2, :], in_=qnT[:])
            nc.sync.dma_start(out=qnT2[H2:128, :], in_=qnT[:])

            ws3d = ws_aug[:].rearrange("p (g c) -> p g c", c=H2 + 1)
            npair_r = nbs // 2
            with (
                tc.tile_pool(name="ph2", bufs=3) as epool,
                tc.tile_pool(name="ps_sim", bufs=3, space="PSUM") as ps_sim,
                tc.tile_pool(name="ps_feat", bufs=1, space="PSUM") as ps_feat,
            ):
                # unpack gathered descriptors; snT packed even->p0:64, odd->p64:128
                for r in range(NCORES):
                    base = r * ccn
                    src = cc_out[base:base + cca].rearrange(
                        "(p q two f) -> p q two f", p=H2, two=2, f=128)
                    dst_e = snT_all[0:H2, :].rearrange(
                        "p (q f) -> p q f", f=128)[:, r * npair_r:(r + 1) * npair_r, :]
                    dst_o = snT_all[H2:128, :].rearrange(
                        "p (q f) -> p q f", f=128)[:, r * npair_r:(r + 1) * npair_r, :]
                    nc.sync.dma_start(out=dst_e, in_=src[:, :, 0, :])
                    nc.sync.dma_start(out=dst_o, in_=src[:, :, 1, :])
                    wsun = epool.tile([128, nbs * H2], f32, tag="wsun")
                    nc.sync.dma_start(
                        out=wsun[:].rearrange("p (b d) -> p b d", d=H2),
                        in_=cc_out[base + cca:base + ccn]
                            .rearrange("(b p d) -> p b d", p=128, d=H2))
                    nc.vector.tensor_copy(
                        ws3d[:, r * nbs:(r + 1) * nbs, 0:H2],
                        wsun[:].rearrange("p (b d) -> p b d", d=H2))
                nc.vector.memset(ws3d[:, :, H2:H2 + 1], 1.0)

                featT = ps_feat.tile([H2 + 1, b_q], f32, tag="feat")
                for pg in range(nblk // 2):
                    sim = ps_sim.tile([128, 2 * b_q], f32, tag="sim")
                    # critical section: keep the two row-strip matmuls adjacent
                    # on PE so they actually run concurrently
                    with tc.tile_critical():
                        nc.tensor.matmul(
                            sim[:, 0:b_q],
                            snT_all[0:H2, pg * 128:(pg + 1) * 128], qnT2[0:H2, :],
                            start=True, stop=True, tile_position=(0, 0))
                        nc.tensor.matmul(
                            sim[:, b_q:2 * b_q],
                            snT_all[H2:128, pg * 128:(pg + 1) * 128], qnT2[H2:128, :],
                            start=True, stop=True, tile_position=(64, 0))
                    eT = epool.tile([128, 2 * b_q], bf16, tag="eT")
                    nc.scalar.activation(eT[:], sim[:], AF.Exp)
                    for j in range(2):
                        gb = 2 * pg + j
                        nc.tensor.matmul(
                            featT[:], ws3d[:, gb, :], eT[:, j * b_q:(j + 1) * b_q],
                            start=(gb == 0), stop=(gb == nblk - 1),
                            skip_group_check=True)

                # DVE is lane-locked: stage the sums row on its own partition,
                # then DMA it (address-based) down to partition 0.
                sums65 = ppool.tile([H2 + 1, b_q], f32, tag="sums65")
                nc.vector.tensor_copy(sums65[H2:H2 + 1, :], featT[H2:H2 + 1, :])
                nc.sync.dma_start(out=sums[:], in_=sums65[H2:H2 + 1, :])
                nc.vector.tensor_copy(featA[:], featT[0:H2, :])

            with tc.tile_pool(name="ps_pred", bufs=1, space="PSUM") as ps_pred:
                zt = ps_pred.tile([H4, b_q], f32, tag="zt")
                nc.tensor.matmul(zt[:], wp1[:], featA[:], start=True, stop=False)
                nc.tensor.matmul(zt[:], bp1r[:], sums[:], start=False, stop=True)
                aT = ppool.tile([H4, b_q], f32, tag="aT")
                nc.vector.tensor_scalar_max(aT[:], zt[:], 0.0)

                pred = ps_pred.tile([1, b_q], f32, tag="pred")
                nc.tensor.matmul(pred[:], wp2[:], aT[:], start=True, stop=True)
                recip = ppool.tile([1, b_q], f32, tag="recip")
                nc.vector.reciprocal(recip[:], sums[:])
                ptmp = ppool.tile([1, b_q], f32, tag="ptmp")
                nc.vector.tensor_tensor(out=ptmp[:], in0=pred[:], in1=recip[:],
                                        op=OP.mult)
                yrow = ppool.tile([1, b_q], f32, tag="yrow")
                nc.vector.tensor_scalar(yrow[:], ptmp[:], svec[:, 0:1], None,
                                        OP.add)
                nc.sync.dma_start(out=y_d[:, :].rearrange("a b -> b a"), in_=yrow[:])

    return nc


def _get_nc(b_q=BQ, s_l=SL):
    key = (b_q, s_l)
    if key not in _CACHE:
        nc = _build(b_q, s_l)
        nc.finalize()
        _CACHE[key] = nc
    return _CACHE[key]


LAST_RESULTS = None


def kernel(x, support_set, params, _trace=False):
    global LAST_RESULTS
    from concourse.bass_utils import run_bass_kernel_spmd

    x = np.asarray(x, np.float32)
    support_set = np.asarray(support_set, np.float32)
    folded = _fold(params)
    xT = np.asarray(x.T, np.float32)
    sT = np.asarray(support_set.T, np.float32)

    nc = _get_nc()

    common = dict(
        w1=folded["W1"],
        w2=folded["w2"], w3=folded["w3"],
        wa1=folded["wa1"], wa2=folded["wa2"],
        wp1=folded["wp1"], wp2=folded["wp2"],
        vecs=folded["vecs"], bp1r=folded["bp1r"], svec=folded["svec"],
    )
    in_maps = []
    for r in range(NCORES):
        in_maps.append(dict(
            xqT=np.ascontiguousarray(xT[:, r * BQ:(r + 1) * BQ]),
            xsT=np.ascontiguousarray(sT[:, r * SL:(r + 1) * SL]),
            **common,
        ))

    kw = {}
    if _trace:
        kw = dict(trace=True, trace_cores=list(range(NCORES)))
    res = run_bass_kernel_spmd(nc, in_maps, list(range(NCORES)), **kw)
    LAST_RESULTS = res
    y = np.concatenate([res.results[r]["y"] for r in range(NCORES)], axis=0)
    return y.astype(np.float32)


# revision 36
# speedup vs baseline: 1.6164x; 1.6164x over previous
"""DN4++ retrieval kernel for 8 Trainium2 NeuronCores.

Strategy
--------
Math: with eval-mode BN folded into the linear layers, the backbone is
    a1 = relu(x @ W1 + b1); a2 = relu(a1 @ W2f + b2f); a3 = relu(a2 @ W3f + b3f)
    d  = s3*a3 + t3                      (bn3 affine)
    att = sigmoid(a3 @ Wa1f + ba1f -> relu -> @ Wa2 + ba2)
    w   = d * att
The attention factor cancels inside the cosine normalization (att > 0), so
    qn = d_q/||d_q||, sn = d_s/||d_s||, and att is only needed for the
support-value rows ws = d_s*att_s.  Softmax skips max-subtraction
(logits in [-10,10]) and the denominator is obtained from a ones-column in
the feat matmul; the division is deferred through the predictor's relu
(positive homogeneity).

Sharding: support MLP is sharded (1024 rows/core); the tiny descriptors
(snT pre-scaled by 10/||d_s||, and ws rows) are AllGathered; queries are
data-parallel (512/core).  Everything runs in one SPMD launch.
"""

import numpy as np

NCORES = 8
B, S, DIN, H, H2, H4 = 4096, 8192, 512, 128, 64, 32
BQ = B // NCORES          # 512 queries per core
SL = S // NCORES          # 1024 support rows per core
NBQ = BQ // 128           # 4 query blocks / core
NBS = SL // 128           # 8 support blocks / core
NBLK = S // 128           # 64 global support blocks
KC = DIN // 128           # 4 contraction chunks for layer 1
CCA = H2 * SL             # floats of snT (pre-scaled) per rank
CCB = SL * H2             # floats of ws rows per rank
CCN = CCA + CCB           # gathered floats per rank
BN_EPS = 1e-5
TEMP = 10.0
GRP = 3                   # support blocks per exp group (3 psum banks)

_CACHE: dict = {}


def _split_hi_lo(a):
    import ml_dtypes
    a = np.ascontiguousarray(np.asarray(a, np.float32))
    hi = a.astype(ml_dtypes.bfloat16)
    lo = (a - hi.astype(np.float32)).astype(ml_dtypes.bfloat16)
    return hi, lo


def _fold(params):
    g = lambda a: np.asarray(a, np.float64)

    def bn_affine(bn):
        s = g(bn["g"]) / np.sqrt(g(bn["v"]) + BN_EPS)
        t = g(bn["b"]) - g(bn["m"]) * s
        return s, t

    s1, t1 = bn_affine(params["bn1"])
    s2, t2 = bn_affine(params["bn2"])
    s3, t3 = bn_affine(params["bn3"])
    snp, tnp = bn_affine(params["bnp"])

    W1 = g(params["W1"])
    b1 = g(params["b1"])
    W2f = s1[:, None] * g(params["W2"])
    b2f = g(params["b2"]) + t1 @ g(params["W2"])
    W3f = s2[:, None] * g(params["W3"])
    b3f = g(params["b3"]) + t2 @ g(params["W3"])
    Wa1f = s3[:, None] * g(params["Wa1"])
    ba1f = g(params["ba1"]) + t3 @ g(params["Wa1"])
    Wa2 = g(params["Wa2"])
    ba2 = float(np.reshape(g(params["ba2"]), -1)[0])
    Wp1 = g(params["Wp1"])
    bp1 = g(params["bp1"])
    Wp2f = snp[:, None] * g(params["Wp2"])
    bp2f = float(np.reshape(g(params["bp2"]) + tnp @ g(params["Wp2"]), -1)[0])

    f = lambda a: np.ascontiguousarray(np.asarray(a, np.float32))
    vecs = np.zeros((H, 8), np.float32)
    vecs[:, 0] = f(b1)
    vecs[:, 1] = f(b2f)
    vecs[:H2, 2] = f(b3f)
    vecs[:H2, 3] = f(s3)
    vecs[:H2, 4] = f(t3)
    vecs[:H4, 5] = f(ba1f)
    vecs[:, 7] = -ba2            # bias for exp(-z - ba2)

    return dict(
        W1=f(W1), vecs=vecs,
        w2=f(W2f), w3=f(W3f), wa1=f(Wa1f), wa2=f(Wa2),
        wp1=f(Wp1), wp2=f(Wp2f),
        bp1r=f(bp1).reshape(1, H4),
        svec=np.array([[bp2f, 0.0]], np.float32),
    )


def _build(b_q=BQ, s_l=SL, for_sim=False):
    """Build the SPMD Bass program (shape-dependent only)."""
    import concourse.bass as bass
    import concourse.mybir as mybir
    from concourse import masks
    from concourse.tile import TileContext

    dt = mybir.dt
    f32 = dt.float32
    bf16 = dt.bfloat16
    AF = mybir.ActivationFunctionType
    OP = mybir.AluOpType

    nbq = b_q // 128
    nbs = s_l // 128
    nblk = nbs * NCORES
    cca = H2 * s_l
    ccb = s_l * H2
    ccn = cca + ccb
    LN10 = float(np.log(10.0))

    from concourse import bacc

    # Pin Exp/Ln/Square to the one ACT table set containing all three, so the
    # compiler never alternates table loads (~2.7us per switch). Set ids are
    # positional, so preserve dict size/order and only strip functions.
    if not getattr(bacc, "_act_tables_pinned", False):
        import concourse.hw_specs as hw_specs
        _orig_tables = hw_specs.get_activation_tables
        AFT = mybir.ActivationFunctionType
        _strip = {AFT.Exp, AFT.Ln, AFT.Square}
        _target = "natural_log_exp_and_others"

        def _pinned_tables(arch, _orig=_orig_tables):
            full = _orig(arch)
            return {name: (funcs if name == _target else funcs - _strip)
                    for name, funcs in full.items()}

        bacc.get_activation_tables = _pinned_tables
        bacc._act_tables_pinned = True

    if for_sim:
        nc = bacc.Bacc(None, target_bir_lowering=False, debug=True)
    else:
        nc = bacc.Bacc(None)

    # ---------------- I/O ----------------
    # x is pre-transposed on the host: contraction dim on partitions.
    xqT_d = nc.declare_dram_parameter("xqT", [DIN, b_q], f32, isOutput=False)
    xsT_d = nc.declare_dram_parameter("xsT", [DIN, s_l], f32, isOutput=False)
    w1_d = nc.declare_dram_parameter("w1", [DIN, H], f32, isOutput=False)
    w2_d = nc.declare_dram_parameter("w2", [H, H], f32, isOutput=False)
    w3_d = nc.declare_dram_parameter("w3", [H, H2], f32, isOutput=False)
    wa1_d = nc.declare_dram_parameter("wa1", [H2, H4], f32, isOutput=False)
    wa2_d = nc.declare_dram_parameter("wa2", [H4, 1], f32, isOutput=False)
    wp1_d = nc.declare_dram_parameter("wp1", [H2, H4], f32, isOutput=False)
    wp2_d = nc.declare_dram_parameter("wp2", [H4, 1], f32, isOutput=False)
    vecs_d = nc.declare_dram_parameter("vecs", [H, 8], f32, isOutput=False)
    bp1r_d = nc.declare_dram_parameter("bp1r", [1, H4], f32, isOutput=False)
    svec_d = nc.declare_dram_parameter("svec", [1, 2], f32, isOutput=False)
    y_d = nc.declare_dram_parameter("y", [b_q, 1], f32, isOutput=True)

    # internal DRAM (descriptors travel as bf16)
    cc_in = nc.dram_tensor("cc_in", [ccn], bf16)
    cc_out = nc.dram_tensor("cc_out", [NCORES * ccn], bf16, addr_space="Shared")

    with TileContext(nc) as tc:
        with (
            tc.tile_pool(name="const", bufs=1) as cpool,
            tc.tile_pool(name="persist", bufs=1) as ppool,
        ):
            # ---------------- constants ----------------
            w1 = cpool.tile([128, KC * H], f32, tag="w1")
            nc.sync.dma_start(
                out=w1[:].rearrange("p (c m) -> p c m", m=H),
                in_=w1_d[:, :].rearrange("(c p) m -> p c m", p=128),
            )
            w2 = cpool.tile([H, H], f32, tag="w2")
            nc.sync.dma_start(out=w2[:], in_=w2_d[:, :])
            w3 = cpool.tile([H, H2], f32, tag="w3")
            nc.sync.dma_start(out=w3[:], in_=w3_d[:, :])
            wa1 = cpool.tile([H2, H4], f32, tag="wa1")
            nc.sync.dma_start(out=wa1[:], in_=wa1_d[:, :])
            wa2 = cpool.tile([H4, 1], f32, tag="wa2")
            nc.sync.dma_start(out=wa2[:], in_=wa2_d[:, :])
            wp1 = cpool.tile([H2, H4], f32, tag="wp1")
            nc.sync.dma_start(out=wp1[:], in_=wp1_d[:, :])
            wp2 = cpool.tile([H4, 1], f32, tag="wp2")
            nc.sync.dma_start(out=wp2[:], in_=wp2_d[:, :])
            vecs = cpool.tile([H, 8], f32, tag="vecs")
            nc.sync.dma_start(out=vecs[:], in_=vecs_d[:, :])
            bp1r = cpool.tile([1, H4], f32, tag="bp1r")
            nc.sync.dma_start(out=bp1r[:], in_=bp1r_d[:, :])
            svec = cpool.tile([1, 2], f32, tag="svec")
            nc.sync.dma_start(out=svec[:], in_=svec_d[:, :])

            id128 = cpool.tile([128, 128], f32, tag="id128")
            masks.make_identity(nc, id128[:])
            ones1 = cpool.tile([1, H2], f32, tag="ones1")
            nc.vector.memset(ones1[:], 1.0)
            v_ln10 = cpool.tile([128, 1], f32, tag="ln10")
            nc.vector.memset(v_ln10[:], LN10)

            v_b1 = vecs[:, 0:1]
            v_b2 = vecs[:, 1:2]
            v_b3 = vecs[0:H2, 2:3]
            v_s3 = vecs[0:H2, 3:4]
            v_t3 = vecs[0:H2, 4:5]
            v_ba1 = vecs[0:H4, 5:6]
            v_nba2 = vecs[:, 7:8]

            # ---------------- persistent tiles ----------------
            dsT = ppool.tile([H2, s_l], f32, tag="dsT")       # raw support desc, T
            dqT = ppool.tile([H2, b_q], f32, tag="dqT")
            snTs = ppool.tile([H2, s_l], bf16, tag="snTs")    # 10/||d|| * dsT
            ws_l = ppool.tile([128, nbs * H2], bf16, tag="ws_l")
            ss_s = ppool.tile([128, nbs], f32, tag="ss_s")
            ss_q = ppool.tile([128, nbq], f32, tag="ss_q")
            attz_all = ppool.tile([128, nbs], f32, tag="attz_all")
            att_all = ppool.tile([128, nbs], f32, tag="att_all")
            s10r = ppool.tile([128, nbs], f32, tag="s10r")
            qnT = ppool.tile([H2, b_q], bf16, tag="qnT")
            qnT2 = ppool.tile([128, b_q], bf16, tag="qnT2")
            scale_row = ppool.tile([1, s_l], f32, tag="scale_row")
            # packed layout: even global blocks on partitions 0-63, odd on
            # 64-127; pair p occupies columns p*128:(p+1)*128
            snT_all = ppool.tile([128, nblk * 64], bf16, tag="snT_all")
            ws_stage = ppool.tile([128, nbs * H2], f32, tag="ws_stage")
            ws_aug = ppool.tile([128, nblk * (H2 + 1)], bf16, tag="ws_aug")
            sums = ppool.tile([1, b_q], f32, tag="sums")
            featA = ppool.tile([H2, b_q], f32, tag="featA")

            # =============== PHASE 1 ===============
            with (
                tc.tile_pool(name="ph1", bufs=2) as wpool,
                tc.tile_pool(name="ph1x", bufs=4) as xpool,
                tc.tile_pool(name="ps_mlp", bufs=3, space="PSUM") as ps_mlp,
                tc.tile_pool(name="ps_msc", bufs=1, space="PSUM") as ps_msc,
                tc.tile_pool(name="ps_drow", bufs=2, space="PSUM") as ps_drow,
            ):

                def mlp_tile(srcT, r0, R, dT_out):
                    """Backbone for rows [r0, r0+R); writes dT (post-bn3) into
                    dT_out AP ([H2, R]) and returns a3T tile for attention."""
                    h1 = ps_mlp.tile([128, R], f32, tag="mlp")
                    for k in range(KC):
                        xt = xpool.tile([128, R], f32, tag="xt")
                        nc.sync.dma_start(
                            out=xt[:], in_=srcT[k * 128:(k + 1) * 128, r0:r0 + R])
                        wk = w1[:].rearrange("p (c m) -> p c m", m=H)[:, k, :]
                        nc.tensor.matmul(h1[:], wk, xt[:],
                                         start=(k == 0), stop=(k == KC - 1))
                    a1 = wpool.tile([128, R], f32, tag="a1")
                    nc.vector.tensor_scalar(a1[:], h1[:], v_b1, 0.0, OP.add, OP.max)

                    h2 = ps_mlp.tile([128, R], f32, tag="mlp")
                    nc.tensor.matmul(h2[:], w2[:], a1[:], start=True, stop=True)
                    a2 = wpool.tile([128, R], f32, tag="a2")
                    nc.vector.tensor_scalar(a2[:], h2[:], v_b2, 0.0, OP.add, OP.max)

                    h3 = ps_msc.tile([H2, R], f32, tag="misc1")
                    nc.tensor.matmul(h3[:], w3[:], a2[:], start=True, stop=True)
                    a3 = wpool.tile([H2, R], f32, tag="a3")
                    nc.vector.tensor_scalar(a3[:], h3[:], v_b3, 0.0, OP.add, OP.max)

                    # d = s3*a3 + t3   (transposed layout: per-partition affine)
                    nc.vector.tensor_scalar(dT_out, a3[:], v_s3, v_t3, OP.mult, OP.add)
                    return a3

                # ---- support tiles ----
                RT = min(512, s_l)
                for t in range(s_l // RT):
                    dT_out = dsT[:, t * RT:(t + 1) * RT]
                    a3 = mlp_tile(xsT_d, t * RT, RT, dT_out)

                    # attention branch
                    z1 = ps_msc.tile([H4, RT], f32, tag="misc1")
                    nc.tensor.matmul(z1[:], wa1[:], a3[:], start=True, stop=True)
                    za1 = wpool.tile([H4, RT], f32, tag="za1")
                    nc.vector.tensor_scalar(za1[:], z1[:], v_ba1, 0.0, OP.add, OP.max)

                    nb = RT // 128
                    attz = ps_msc.tile([128, nb], f32, tag="misc1")
                    for j in range(nb):
                        nc.tensor.matmul(attz[:, j:j + 1],
                                         za1[:, j * 128:(j + 1) * 128], wa2[:],
                                         start=True, stop=True)
                    nc.vector.tensor_copy(attz_all[:, t * nb:(t + 1) * nb], attz[:])

                    drow = ps_drow.tile([128, nb * H2], f32, tag="drow")
                    scr64 = wpool.tile([128, H2], f32, tag="scr64")
                    for j in range(nb):
                        blk = t * nb + j
                        dblk = drow[:, j * H2:(j + 1) * H2]
                        nc.tensor.transpose(
                            dblk, dT_out[:, j * 128:(j + 1) * 128], id128[0:H2, 0:H2])
                        nc.scalar.activation(scr64[:], dblk, AF.Square,
                                             accum_out=ss_s[:, blk:blk + 1])
                    # stage raw d rows; the att multiply happens after the
                    # (batched) transcendental block below
                    nc.vector.tensor_copy(
                        ws_stage[:, t * nb * H2:(t + 1) * nb * H2], drow[:])

                # batched transcendentals (single ACT table set):
                # scale10r = 10/||d_s|| = exp(-0.5*ln(ss) + ln 10)
                lnss = wpool.tile([128, nbs], f32, tag="lnss")
                nc.scalar.activation(lnss[:], ss_s[:], AF.Ln)
                nc.scalar.activation(s10r[:], lnss[:], AF.Exp, bias=v_ln10[:],
                                     scale=-0.5)
                # att = sigmoid(z + ba2) = 1/(1 + exp(-(z + ba2)))
                enz = wpool.tile([128, nbs], f32, tag="enz")
                nc.scalar.activation(enz[:], attz_all[:], AF.Exp, bias=v_nba2,
                                     scale=-1.0)
                ra = wpool.tile([128, nbs], f32, tag="ra")
                nc.vector.tensor_scalar_add(ra[:], enz[:], 1.0)
                nc.vector.reciprocal(att_all[:], ra[:])
                for b in range(nbs):
                    nc.vector.tensor_scalar_mul(
                        ws_l[:, b * H2:(b + 1) * H2],
                        ws_stage[:, b * H2:(b + 1) * H2], att_all[:, b:b + 1])

                # flatten [128, nbs] -> row vector [1, s_l] with per-block
                # SBUF->SBUF DMAs (address-based, cross partitions)
                for b in range(nbs):
                    nc.sync.dma_start(
                        out=scale_row[:, b * 128:(b + 1) * 128],
                        in_=s10r[:, b:b + 1])

                # snTs = dsT * bcast(scale_row)
                CW = min(512, s_l)
                for c in range(s_l // CW):
                    bc = ps_msc.tile([H2, CW], f32, tag="bq")
                    nc.tensor.matmul(bc[:], ones1[:],
                                     scale_row[:, c * CW:(c + 1) * CW],
                                     start=True, stop=True)
                    nc.vector.tensor_tensor(
                        out=snTs[:, c * CW:(c + 1) * CW],
                        in0=dsT[:, c * CW:(c + 1) * CW], in1=bc[:], op=OP.mult)

                # pack + AllGather (both p-major, contiguous per partition)
                nc.sync.dma_start(
                    out=cc_in[0:cca].rearrange("(p f) -> p f", p=H2), in_=snTs[:])
                nc.sync.dma_start(
                    out=cc_in[cca:ccn].rearrange("(p x) -> p x", p=128),
                    in_=ws_l[:])
                nc.gpsimd.collective_compute(
                    "AllGather",
                    mybir.AluOpType.bypass,
                    replica_groups=[list(range(NCORES))],
                    ins=[cc_in[:]],
                    outs=[cc_out[:]],
                )

                # ---- query tile(s) (overlaps the AllGather) ----
                RTQ = min(512, b_q)
                for tq in range(b_q // RTQ):
                    dT_out = dqT[:, tq * RTQ:(tq + 1) * RTQ]
                    mlp_tile(xqT_d, tq * RTQ, RTQ, dT_out)

                    nb = RTQ // 128
                    drow = ps_drow.tile([128, nb * H2], f32, tag="drow")
                    scr64 = wpool.tile([128, H2], f32, tag="scr64")
                    for j in range(nb):
                        blk = tq * nb + j
                        dblk = drow[:, j * H2:(j + 1) * H2]
                        nc.tensor.transpose(
                            dblk, dT_out[:, j * 128:(j + 1) * 128], id128[0:H2, 0:H2])
                        nc.scalar.activation(scr64[:], dblk, AF.Square,
                                             accum_out=ss_q[:, blk:blk + 1])
                    # r_q = exp(-0.5 ln ss)  (no temperature on query side)
                    lnq = wpool.tile([128, nb], f32, tag="lnq")
                    nc.scalar.activation(lnq[:], ss_q[:, tq * nb:(tq + 1) * nb], AF.Ln)
                    rq = wpool.tile([128, nb], f32, tag="rq")
                    nc.scalar.activation(rq[:], lnq[:], AF.Exp, scale=-0.5)
                    for j in range(nb):
                        blk = tq * nb + j
                        dblk = drow[:, j * H2:(j + 1) * H2]
                        qnr = wpool.tile([128, H2], f32, tag="qnr")
                        nc.vector.tensor_scalar_mul(qnr[:], dblk, rq[:, j:j + 1])
                        qb = ps_msc.tile([H2, 128], f32, tag="bq")
                        nc.tensor.transpose(qb[:], qnr[:], id128[:])
                        nc.vector.tensor_copy(qnT[:, blk * 128:(blk + 1) * 128], qb[:])

                # replicate qnT to both partition halves (packed sim row-strips)
                nc.sync.dma_start(out=qnT2[0:H2, :], in_=qnT[:])
                nc.sync.dma_start(out=qnT2[H2:128, :], in_=qnT[:])

            # =============== PHASE 2 ===============
            ws3d = ws_aug[:].rearrange("p (g c) -> p g c", c=H2 + 1)
            npair_r = nbs // 2
            with (
                tc.tile_pool(name="ph2", bufs=3) as epool,
                tc.tile_pool(name="ps_sim", bufs=3, space="PSUM") as ps_sim,
                tc.tile_pool(name="ps_feat", bufs=1, space="PSUM") as ps_feat,
            ):
                # unpack gathered descriptors (per-rank 3D DMAs, direct bf16);
                # snT packed even->p0:64, odd->p64:128
                for r in range(NCORES):
                    base = r * ccn
                    s_src = cc_out[base:base + cca].rearrange(
                        "(p q two f) -> p q two f", p=H2, two=2, f=128)
                    s_dst = snT_all[:].rearrange(
                        "p (q f) -> p q f", f=128)[:, r * npair_r:(r + 1) * npair_r, :]
                    nc.sync.dma_start(out=s_dst[0:H2], in_=s_src[:, :, 0, :])
                    nc.sync.dma_start(out=s_dst[H2:128], in_=s_src[:, :, 1, :])
                    nc.sync.dma_start(
                        out=ws3d[:, r * nbs:(r + 1) * nbs, 0:H2],
                        in_=cc_out[base + cca:base + ccn]
                            .rearrange("(p b d) -> p b d", p=128, d=H2))
                nc.vector.memset(ws3d[:, :, H2:H2 + 1], 1.0)

                featT = ps_feat.tile([H2 + 1, b_q], f32, tag="feat")
                for pg in range(nblk // 2):
                    sim = ps_sim.tile([128, 2 * b_q], f32, tag="sim")
                    nc.tensor.matmul(
                        sim[:, 0:b_q],
                        snT_all[0:H2, pg * 128:(pg + 1) * 128], qnT2[0:H2, :],
                        start=True, stop=True, tile_position=(0, 0))
                    nc.tensor.matmul(
                        sim[:, b_q:2 * b_q],
                        snT_all[H2:128, pg * 128:(pg + 1) * 128], qnT2[H2:128, :],
                        start=True, stop=True, tile_position=(64, 0))
                    eT = epool.tile([128, 2 * b_q], bf16, tag="eT")
                    nc.scalar.activation(eT[:], sim[:], AF.Exp)
                    for j in range(2):
                        gb = 2 * pg + j
                        nc.tensor.matmul(
                            featT[:], ws3d[:, gb, :], eT[:, j * b_q:(j + 1) * b_q],
                            start=(gb == 0), stop=(gb == nblk - 1),
                            skip_group_check=True)

                # DVE is lane-locked: stage the sums row on its own partition,
                # then DMA it (address-based) down to partition 0.
                sums65 = ppool.tile([H2 + 1, b_q], f32, tag="sums65")
                nc.vector.tensor_copy(sums65[H2:H2 + 1, :], featT[H2:H2 + 1, :])
                nc.sync.dma_start(out=sums[:], in_=sums65[H2:H2 + 1, :])
                nc.vector.tensor_copy(featA[:], featT[0:H2, :])

            with tc.tile_pool(name="ps_pred", bufs=1, space="PSUM") as ps_pred:
                zt = ps_pred.tile([H4, b_q], f32, tag="zt")
                nc.tensor.matmul(zt[:], wp1[:], featA[:], start=True, stop=False)
                nc.tensor.matmul(zt[:], bp1r[:], sums[:], start=False, stop=True)
                aT = ppool.tile([H4, b_q], f32, tag="aT")
                nc.vector.tensor_scalar_max(aT[:], zt[:], 0.0)

                pred = ps_pred.tile([1, b_q], f32, tag="pred")
                nc.tensor.matmul(pred[:], wp2[:], aT[:], start=True, stop=True)
                recip = ppool.tile([1, b_q], f32, tag="recip")
                nc.vector.reciprocal(recip[:], sums[:])
                ptmp = ppool.tile([1, b_q], f32, tag="ptmp")
                nc.vector.tensor_tensor(out=ptmp[:], in0=pred[:], in1=recip[:],
                                        op=OP.mult)
                yrow = ppool.tile([1, b_q], f32, tag="yrow")
                nc.vector.tensor_scalar(yrow[:], ptmp[:], svec[:, 0:1], None,
                                        OP.add)
                nc.sync.dma_start(out=y_d[:, :].rearrange("a b -> b a"), in_=yrow[:])

    return nc


def _get_nc(b_q=BQ, s_l=SL):
    key = (b_q, s_l)
    if key not in _CACHE:
        nc = _build(b_q, s_l)
        nc.finalize()
        _CACHE[key] = nc
    return _CACHE[key]


LAST_RESULTS = None


def kernel(x, support_set, params, _trace=False):
    global LAST_RESULTS
    from concourse.bass_utils import run_bass_kernel_spmd

    x = np.asarray(x, np.float32)
    support_set = np.asarray(support_set, np.float32)
    folded = _fold(params)
    xT = np.asarray(x.T, np.float32)
    sT = np.asarray(support_set.T, np.float32)

    nc = _get_nc()

    common = dict(
        w1=folded["W1"],
        w2=folded["w2"], w3=folded["w3"],
        wa1=folded["wa1"], wa2=folded["wa2"],
        wp1=folded["wp1"], wp2=folded["wp2"],
        vecs=folded["vecs"], bp1r=folded["bp1r"], svec=folded["svec"],
    )
    in_maps = []
    for r in range(NCORES):
        in_maps.append(dict(
            xqT=np.ascontiguousarray(xT[:, r * BQ:(r + 1) * BQ]),
            xsT=np.ascontiguousarray(sT[:, r * SL:(r + 1) * SL]),
            **common,
        ))

    kw = {}
    if _trace:
        kw = dict(trace=True, trace_cores=list(range(NCORES)))
    res = run_bass_kernel_spmd(nc, in_maps, list(range(NCORES)), **kw)
    LAST_RESULTS = res
    y = np.concatenate([res.results[r]["y"] for r in range(NCORES)], axis=0)
    return y.astype(np.float32)


# revision 38
# speedup vs baseline: 1.7337x; 1.0726x over previous
"""DN4++ retrieval kernel for 8 Trainium2 NeuronCores.

Strategy
--------
Math: with eval-mode BN folded into the linear layers, the backbone is
    a1 = relu(x @ W1 + b1); a2 = relu(a1 @ W2f + b2f); a3 = relu(a2 @ W3f + b3f)
    d  = s3*a3 + t3                      (bn3 affine)
    att = sigmoid(a3 @ Wa1f + ba1f -> relu -> @ Wa2 + ba2)
    w   = d * att
The attention factor cancels inside the cosine normalization (att > 0), so
    qn = d_q/||d_q||, sn = d_s/||d_s||, and att is only needed for the
support-value rows ws = d_s*att_s.  Softmax skips max-subtraction
(logits in [-10,10]) and the denominator is obtained from a ones-column in
the feat matmul; the division is deferred through the predictor's relu
(positive homogeneity).

Sharding: support MLP is sharded (1024 rows/core); the tiny descriptors
(snT pre-scaled by 10/||d_s||, and ws rows) are AllGathered; queries are
data-parallel (512/core).  Everything runs in one SPMD launch.
"""

import numpy as np

NCORES = 8
B, S, DIN, H, H2, H4 = 4096, 8192, 512, 128, 64, 32
BQ = B // NCORES          # 512 queries per core
SL = S // NCORES          # 1024 support rows per core
NBQ = BQ // 128           # 4 query blocks / core
NBS = SL // 128           # 8 support blocks / core
NBLK = S // 128           # 64 global support blocks
KC = DIN // 128           # 4 contraction chunks for layer 1
CCA = H2 * SL             # floats of snT (pre-scaled) per rank
CCB = SL * H2             # floats of ws rows per rank
CCN = CCA + CCB           # gathered floats per rank
BN_EPS = 1e-5
TEMP = 10.0
GRP = 3                   # support blocks per exp group (3 psum banks)

_CACHE: dict = {}


def _split_hi_lo(a):
    import ml_dtypes
    a = np.ascontiguousarray(np.asarray(a, np.float32))
    hi = a.astype(ml_dtypes.bfloat16)
    lo = (a - hi.astype(np.float32)).astype(ml_dtypes.bfloat16)
    return hi, lo


def _fold(params):
    g = lambda a: np.asarray(a, np.float64)

    def bn_affine(bn):
        s = g(bn["g"]) / np.sqrt(g(bn["v"]) + BN_EPS)
        t = g(bn["b"]) - g(bn["m"]) * s
        return s, t

    s1, t1 = bn_affine(params["bn1"])
    s2, t2 = bn_affine(params["bn2"])
    s3, t3 = bn_affine(params["bn3"])
    snp, tnp = bn_affine(params["bnp"])

    W1 = g(params["W1"])
    b1 = g(params["b1"])
    W2f = s1[:, None] * g(params["W2"])
    b2f = g(params["b2"]) + t1 @ g(params["W2"])
    W3f = s2[:, None] * g(params["W3"])
    b3f = g(params["b3"]) + t2 @ g(params["W3"])
    Wa1f = s3[:, None] * g(params["Wa1"])
    ba1f = g(params["ba1"]) + t3 @ g(params["Wa1"])
    Wa2 = g(params["Wa2"])
    ba2 = float(np.reshape(g(params["ba2"]), -1)[0])
    Wp1 = g(params["Wp1"])
    bp1 = g(params["bp1"])
    Wp2f = snp[:, None] * g(params["Wp2"])
    bp2f = float(np.reshape(g(params["bp2"]) + tnp @ g(params["Wp2"]), -1)[0])

    import ml_dtypes
    f = lambda a: np.ascontiguousarray(np.asarray(a, np.float32))
    h = lambda a: np.ascontiguousarray(np.asarray(a, np.float32).astype(ml_dtypes.bfloat16))
    vecs = np.zeros((H, 8), np.float32)
    vecs[:, 0] = f(b1)
    vecs[:, 1] = f(b2f)
    vecs[:H2, 2] = f(b3f)
    vecs[:H2, 3] = f(s3)
    vecs[:H2, 4] = f(t3)
    vecs[:H4, 5] = f(ba1f)
    vecs[:, 7] = -ba2            # bias for exp(-z - ba2)

    return dict(
        w1=h(W1), vecs=vecs,
        w2=h(W2f), w3=h(W3f), wa1=h(Wa1f), wa2=h(Wa2),
        wp1=f(Wp1), wp2=f(Wp2f),
        bp1r=f(bp1).reshape(1, H4),
        svec=np.array([[bp2f, 0.0]], np.float32),
    )


def _build(b_q=BQ, s_l=SL, for_sim=False):
    """Build the SPMD Bass program (shape-dependent only)."""
    import concourse.bass as bass
    import concourse.mybir as mybir
    from concourse import masks
    from concourse.tile import TileContext

    dt = mybir.dt
    f32 = dt.float32
    bf16 = dt.bfloat16
    AF = mybir.ActivationFunctionType
    OP = mybir.AluOpType

    nbq = b_q // 128
    nbs = s_l // 128
    nblk = nbs * NCORES
    cca = H2 * s_l
    ccb = s_l * H2
    ccn = cca + ccb
    LN10 = float(np.log(10.0))

    from concourse import bacc

    # Pin Exp/Ln/Square to the one ACT table set containing all three, so the
    # compiler never alternates table loads (~2.7us per switch). Set ids are
    # positional, so preserve dict size/order and only strip functions.
    if not getattr(bacc, "_act_tables_pinned", False):
        import concourse.hw_specs as hw_specs
        _orig_tables = hw_specs.get_activation_tables
        AFT = mybir.ActivationFunctionType
        _strip = {AFT.Exp, AFT.Ln, AFT.Square}
        _target = "natural_log_exp_and_others"

        def _pinned_tables(arch, _orig=_orig_tables):
            full = _orig(arch)
            return {name: (funcs if name == _target else funcs - _strip)
                    for name, funcs in full.items()}

        bacc.get_activation_tables = _pinned_tables
        bacc._act_tables_pinned = True

    if for_sim:
        nc = bacc.Bacc(None, target_bir_lowering=False, debug=True)
    else:
        nc = bacc.Bacc(None)

    # ---------------- I/O ----------------
    # x is pre-transposed on the host: contraction dim on partitions.
    xqT_d = nc.declare_dram_parameter("xqT", [DIN, b_q], bf16, isOutput=False)
    xsT_d = nc.declare_dram_parameter("xsT", [DIN, s_l], bf16, isOutput=False)
    w1_d = nc.declare_dram_parameter("w1", [DIN, H], bf16, isOutput=False)
    w2_d = nc.declare_dram_parameter("w2", [H, H], bf16, isOutput=False)
    w3_d = nc.declare_dram_parameter("w3", [H, H2], bf16, isOutput=False)
    wa1_d = nc.declare_dram_parameter("wa1", [H2, H4], bf16, isOutput=False)
    wa2_d = nc.declare_dram_parameter("wa2", [H4, 1], bf16, isOutput=False)
    wp1_d = nc.declare_dram_parameter("wp1", [H2, H4], f32, isOutput=False)
    wp2_d = nc.declare_dram_parameter("wp2", [H4, 1], f32, isOutput=False)
    vecs_d = nc.declare_dram_parameter("vecs", [H, 8], f32, isOutput=False)
    bp1r_d = nc.declare_dram_parameter("bp1r", [1, H4], f32, isOutput=False)
    svec_d = nc.declare_dram_parameter("svec", [1, 2], f32, isOutput=False)
    y_d = nc.declare_dram_parameter("y", [b_q, 1], f32, isOutput=True)

    # internal DRAM (descriptors travel as bf16)
    cc_in = nc.dram_tensor("cc_in", [ccn], bf16)
    cc_out = nc.dram_tensor("cc_out", [NCORES * ccn], bf16, addr_space="Shared")

    with TileContext(nc) as tc:
        with (
            tc.tile_pool(name="const", bufs=1) as cpool,
            tc.tile_pool(name="persist", bufs=1) as ppool,
        ):
            # ---------------- constants ----------------
            w1 = cpool.tile([128, KC * H], bf16, tag="w1")
            nc.sync.dma_start(
                out=w1[:].rearrange("p (c m) -> p c m", m=H),
                in_=w1_d[:, :].rearrange("(c p) m -> p c m", p=128),
            )
            w2 = cpool.tile([H, H], bf16, tag="w2")
            nc.sync.dma_start(out=w2[:], in_=w2_d[:, :])
            w3 = cpool.tile([H, H2], bf16, tag="w3")
            nc.sync.dma_start(out=w3[:], in_=w3_d[:, :])
            wa1 = cpool.tile([H2, H4], bf16, tag="wa1")
            nc.sync.dma_start(out=wa1[:], in_=wa1_d[:, :])
            wa2 = cpool.tile([H4, 1], bf16, tag="wa2")
            nc.sync.dma_start(out=wa2[:], in_=wa2_d[:, :])
            wp1 = cpool.tile([H2, H4], f32, tag="wp1")
            nc.sync.dma_start(out=wp1[:], in_=wp1_d[:, :])
            wp2 = cpool.tile([H4, 1], f32, tag="wp2")
            nc.sync.dma_start(out=wp2[:], in_=wp2_d[:, :])
            vecs = cpool.tile([H, 8], f32, tag="vecs")
            nc.sync.dma_start(out=vecs[:], in_=vecs_d[:, :])
            bp1r = cpool.tile([1, H4], f32, tag="bp1r")
            nc.sync.dma_start(out=bp1r[:], in_=bp1r_d[:, :])
            svec = cpool.tile([1, 2], f32, tag="svec")
            nc.sync.dma_start(out=svec[:], in_=svec_d[:, :])

            id128 = cpool.tile([128, 128], f32, tag="id128")
            masks.make_identity(nc, id128[:])
            ones1 = cpool.tile([1, H2], f32, tag="ones1")
            nc.vector.memset(ones1[:], 1.0)
            v_ln10 = cpool.tile([128, 1], f32, tag="ln10")
            nc.vector.memset(v_ln10[:], LN10)

            v_b1 = vecs[:, 0:1]
            v_b2 = vecs[:, 1:2]
            v_b3 = vecs[0:H2, 2:3]
            v_s3 = vecs[0:H2, 3:4]
            v_t3 = vecs[0:H2, 4:5]
            v_ba1 = vecs[0:H4, 5:6]
            v_nba2 = vecs[:, 7:8]

            # ---------------- persistent tiles ----------------
            dsT = ppool.tile([H2, s_l], f32, tag="dsT")       # raw support desc, T
            dqT = ppool.tile([H2, b_q], f32, tag="dqT")
            snTs = ppool.tile([H2, s_l], bf16, tag="snTs")    # 10/||d|| * dsT
            ws_l = ppool.tile([128, nbs * H2], bf16, tag="ws_l")
            ss_s = ppool.tile([128, nbs], f32, tag="ss_s")
            ss_q = ppool.tile([128, nbq], f32, tag="ss_q")
            attz_all = ppool.tile([128, nbs], f32, tag="attz_all")
            att_all = ppool.tile([128, nbs], f32, tag="att_all")
            s10r = ppool.tile([128, nbs], f32, tag="s10r")
            qnT = ppool.tile([H2, b_q], bf16, tag="qnT")
            qnT2 = ppool.tile([128, b_q], bf16, tag="qnT2")
            scale_row = ppool.tile([1, s_l], f32, tag="scale_row")
            # packed layout: even global blocks on partitions 0-63, odd on
            # 64-127; pair p occupies columns p*128:(p+1)*128
            snT_all = ppool.tile([128, nblk * 64], bf16, tag="snT_all")
            ws_stage = ppool.tile([128, nbs * H2], f32, tag="ws_stage")
            ws_aug = ppool.tile([128, nblk * (H2 + 1)], bf16, tag="ws_aug")
            sums = ppool.tile([1, b_q], f32, tag="sums")
            featA = ppool.tile([H2, b_q], f32, tag="featA")

            # =============== PHASE 1 ===============
            with (
                tc.tile_pool(name="ph1", bufs=2) as wpool,
                tc.tile_pool(name="ph1x", bufs=4) as xpool,
                tc.tile_pool(name="ps_mlp", bufs=3, space="PSUM") as ps_mlp,
                tc.tile_pool(name="ps_msc", bufs=1, space="PSUM") as ps_msc,
                tc.tile_pool(name="ps_drow", bufs=2, space="PSUM") as ps_drow,
            ):

                def mlp_tile(srcT, r0, R, dT_out):
                    """Backbone for rows [r0, r0+R); writes dT (post-bn3) into
                    dT_out AP ([H2, R]) and returns a3T tile for attention."""
                    h1 = ps_mlp.tile([128, R], f32, tag="mlp")
                    for k in range(KC):
                        xt = xpool.tile([128, R], bf16, tag="xt")
                        nc.sync.dma_start(
                            out=xt[:], in_=srcT[k * 128:(k + 1) * 128, r0:r0 + R])
                        wk = w1[:].rearrange("p (c m) -> p c m", m=H)[:, k, :]
                        nc.tensor.matmul(h1[:], wk, xt[:],
                                         start=(k == 0), stop=(k == KC - 1))
                    a1 = wpool.tile([128, R], bf16, tag="a1")
                    nc.vector.tensor_scalar(a1[:], h1[:], v_b1, 0.0, OP.add, OP.max)

                    h2 = ps_mlp.tile([128, R], f32, tag="mlp")
                    nc.tensor.matmul(h2[:], w2[:], a1[:], start=True, stop=True)
                    a2 = wpool.tile([128, R], bf16, tag="a2")
                    nc.vector.tensor_scalar(a2[:], h2[:], v_b2, 0.0, OP.add, OP.max)

                    h3 = ps_msc.tile([H2, R], f32, tag="misc1")
                    nc.tensor.matmul(h3[:], w3[:], a2[:], start=True, stop=True)
                    a3 = wpool.tile([H2, R], bf16, tag="a3")
                    nc.vector.tensor_scalar(a3[:], h3[:], v_b3, 0.0, OP.add, OP.max)

                    # d = s3*a3 + t3   (transposed layout: per-partition affine)
                    nc.vector.tensor_scalar(dT_out, a3[:], v_s3, v_t3, OP.mult, OP.add)
                    return a3

                # ---- support tiles ----
                RT = min(512, s_l)
                for t in range(s_l // RT):
                    dT_out = dsT[:, t * RT:(t + 1) * RT]
                    a3 = mlp_tile(xsT_d, t * RT, RT, dT_out)

                    # attention branch
                    z1 = ps_msc.tile([H4, RT], f32, tag="misc1")
                    nc.tensor.matmul(z1[:], wa1[:], a3[:], start=True, stop=True)
                    za1 = wpool.tile([H4, RT], bf16, tag="za1")
                    nc.vector.tensor_scalar(za1[:], z1[:], v_ba1, 0.0, OP.add, OP.max)

                    nb = RT // 128
                    attz = ps_msc.tile([128, nb], f32, tag="misc1")
                    for j in range(nb):
                        nc.tensor.matmul(attz[:, j:j + 1],
                                         za1[:, j * 128:(j + 1) * 128], wa2[:],
                                         start=True, stop=True)
                    nc.vector.tensor_copy(attz_all[:, t * nb:(t + 1) * nb], attz[:])

                    drow = ps_drow.tile([128, nb * H2], f32, tag="drow")
                    scr64 = wpool.tile([128, H2], f32, tag="scr64")
                    for j in range(nb):
                        blk = t * nb + j
                        dblk = drow[:, j * H2:(j + 1) * H2]
                        nc.tensor.transpose(
                            dblk, dT_out[:, j * 128:(j + 1) * 128], id128[0:H2, 0:H2])
                        nc.scalar.activation(scr64[:], dblk, AF.Square,
                                             accum_out=ss_s[:, blk:blk + 1])
                    # stage raw d rows; the att multiply happens after the
                    # (batched) transcendental block below
                    nc.vector.tensor_copy(
                        ws_stage[:, t * nb * H2:(t + 1) * nb * H2], drow[:])

                # batched transcendentals (single ACT table set):
                # scale10r = 10/||d_s|| = exp(-0.5*ln(ss) + ln 10)
                lnss = wpool.tile([128, nbs], f32, tag="lnss")
                nc.scalar.activation(lnss[:], ss_s[:], AF.Ln)
                nc.scalar.activation(s10r[:], lnss[:], AF.Exp, bias=v_ln10[:],
                                     scale=-0.5)
                # att = sigmoid(z + ba2) = 1/(1 + exp(-(z + ba2)))
                enz = wpool.tile([128, nbs], f32, tag="enz")
                nc.scalar.activation(enz[:], attz_all[:], AF.Exp, bias=v_nba2,
                                     scale=-1.0)
                ra = wpool.tile([128, nbs], f32, tag="ra")
                nc.vector.tensor_scalar_add(ra[:], enz[:], 1.0)
                nc.vector.reciprocal(att_all[:], ra[:])
                for b in range(nbs):
                    nc.vector.tensor_scalar_mul(
                        ws_l[:, b * H2:(b + 1) * H2],
                        ws_stage[:, b * H2:(b + 1) * H2], att_all[:, b:b + 1])

                # flatten [128, nbs] -> row vector [1, s_l] with per-block
                # SBUF->SBUF DMAs (address-based, cross partitions)
                for b in range(nbs):
                    nc.sync.dma_start(
                        out=scale_row[:, b * 128:(b + 1) * 128],
                        in_=s10r[:, b:b + 1])

                # snTs = dsT * bcast(scale_row)
                CW = min(512, s_l)
                for c in range(s_l // CW):
                    bc = ps_msc.tile([H2, CW], f32, tag="bq")
                    nc.tensor.matmul(bc[:], ones1[:],
                                     scale_row[:, c * CW:(c + 1) * CW],
                                     start=True, stop=True)
                    nc.vector.tensor_tensor(
                        out=snTs[:, c * CW:(c + 1) * CW],
                        in0=dsT[:, c * CW:(c + 1) * CW], in1=bc[:], op=OP.mult)

                # pack + AllGather (both p-major, contiguous per partition)
                nc.sync.dma_start(
                    out=cc_in[0:cca].rearrange("(p f) -> p f", p=H2), in_=snTs[:])
                nc.sync.dma_start(
                    out=cc_in[cca:ccn].rearrange("(p x) -> p x", p=128),
                    in_=ws_l[:])
                nc.gpsimd.collective_compute(
                    "AllGather",
                    mybir.AluOpType.bypass,
                    replica_groups=[list(range(NCORES))],
                    ins=[cc_in[:]],
                    outs=[cc_out[:]],
                )

                # ---- query tile(s) (overlaps the AllGather) ----
                RTQ = min(512, b_q)
                for tq in range(b_q // RTQ):
                    dT_out = dqT[:, tq * RTQ:(tq + 1) * RTQ]
                    mlp_tile(xqT_d, tq * RTQ, RTQ, dT_out)

                    nb = RTQ // 128
                    drow = ps_drow.tile([128, nb * H2], f32, tag="drow")
                    scr64 = wpool.tile([128, H2], f32, tag="scr64")
                    for j in range(nb):
                        blk = tq * nb + j
                        dblk = drow[:, j * H2:(j + 1) * H2]
                        nc.tensor.transpose(
                            dblk, dT_out[:, j * 128:(j + 1) * 128], id128[0:H2, 0:H2])
                        nc.scalar.activation(scr64[:], dblk, AF.Square,
                                             accum_out=ss_q[:, blk:blk + 1])
                    # r_q = exp(-0.5 ln ss)  (no temperature on query side)
                    lnq = wpool.tile([128, nb], f32, tag="lnq")
                    nc.scalar.activation(lnq[:], ss_q[:, tq * nb:(tq + 1) * nb], AF.Ln)
                    rq = wpool.tile([128, nb], f32, tag="rq")
                    nc.scalar.activation(rq[:], lnq[:], AF.Exp, scale=-0.5)
                    for j in range(nb):
                        blk = tq * nb + j
                        dblk = drow[:, j * H2:(j + 1) * H2]
                        qnr = wpool.tile([128, H2], f32, tag="qnr")
                        nc.vector.tensor_scalar_mul(qnr[:], dblk, rq[:, j:j + 1])
                        qb = ps_msc.tile([H2, 128], f32, tag="bq")
                        nc.tensor.transpose(qb[:], qnr[:], id128[:])
                        nc.vector.tensor_copy(qnT[:, blk * 128:(blk + 1) * 128], qb[:])

                # replicate qnT to both partition halves (packed sim row-strips)
                nc.sync.dma_start(out=qnT2[0:H2, :], in_=qnT[:])
                nc.sync.dma_start(out=qnT2[H2:128, :], in_=qnT[:])

            # =============== PHASE 2 ===============
            ws3d = ws_aug[:].rearrange("p (g c) -> p g c", c=H2 + 1)
            npair_r = nbs // 2
            with (
                tc.tile_pool(name="ph2", bufs=3) as epool,
                tc.tile_pool(name="ps_sim", bufs=3, space="PSUM") as ps_sim,
                tc.tile_pool(name="ps_feat", bufs=1, space="PSUM") as ps_feat,
            ):
                # unpack gathered descriptors (per-rank 3D DMAs, direct bf16);
                # snT packed even->p0:64, odd->p64:128
                for r in range(NCORES):
                    base = r * ccn
                    s_src = cc_out[base:base + cca].rearrange(
                        "(p q two f) -> p q two f", p=H2, two=2, f=128)
                    s_dst = snT_all[:].rearrange(
                        "p (q f) -> p q f", f=128)[:, r * npair_r:(r + 1) * npair_r, :]
                    nc.sync.dma_start(out=s_dst[0:H2], in_=s_src[:, :, 0, :])
                    nc.sync.dma_start(out=s_dst[H2:128], in_=s_src[:, :, 1, :])
                    nc.sync.dma_start(
                        out=ws3d[:, r * nbs:(r + 1) * nbs, 0:H2],
                        in_=cc_out[base + cca:base + ccn]
                            .rearrange("(p b d) -> p b d", p=128, d=H2))
                nc.vector.memset(ws3d[:, :, H2:H2 + 1], 1.0)

                featT = ps_feat.tile([H2 + 1, b_q], f32, tag="feat")
                for pg in range(nblk // 2):
                    sim = ps_sim.tile([128, 2 * b_q], f32, tag="sim")
                    nc.tensor.matmul(
                        sim[:, 0:b_q],
                        snT_all[0:H2, pg * 128:(pg + 1) * 128], qnT2[0:H2, :],
                        start=True, stop=True, tile_position=(0, 0))
                    nc.tensor.matmul(
                        sim[:, b_q:2 * b_q],
                        snT_all[H2:128, pg * 128:(pg + 1) * 128], qnT2[H2:128, :],
                        start=True, stop=True, tile_position=(64, 0))
                    eT = epool.tile([128, 2 * b_q], bf16, tag="eT")
                    nc.scalar.activation(eT[:], sim[:], AF.Exp)
                    for j in range(2):
                        gb = 2 * pg + j
                        nc.tensor.matmul(
                            featT[:], ws3d[:, gb, :], eT[:, j * b_q:(j + 1) * b_q],
                            start=(gb == 0), stop=(gb == nblk - 1),
                            skip_group_check=True)

                # DVE is lane-locked: stage the sums row on its own partition,
                # then DMA it (address-based) down to partition 0.
                sums65 = ppool.tile([H2 + 1, b_q], f32, tag="sums65")
                nc.vector.tensor_copy(sums65[H2:H2 + 1, :], featT[H2:H2 + 1, :])
                nc.sync.dma_start(out=sums[:], in_=sums65[H2:H2 + 1, :])
                nc.vector.tensor_copy(featA[:], featT[0:H2, :])

            with tc.tile_pool(name="ps_pred", bufs=1, space="PSUM") as ps_pred:
                zt = ps_pred.tile([H4, b_q], f32, tag="zt")
                nc.tensor.matmul(zt[:], wp1[:], featA[:], start=True, stop=False)
                nc.tensor.matmul(zt[:], bp1r[:], sums[:], start=False, stop=True)
                aT = ppool.tile([H4, b_q], f32, tag="aT")
                nc.vector.tensor_scalar_max(aT[:], zt[:], 0.0)

                pred = ps_pred.tile([1, b_q], f32, tag="pred")
                nc.tensor.matmul(pred[:], wp2[:], aT[:], start=True, stop=True)
                recip = ppool.tile([1, b_q], f32, tag="recip")
                nc.vector.reciprocal(recip[:], sums[:])
                ptmp = ppool.tile([1, b_q], f32, tag="ptmp")
                nc.vector.tensor_tensor(out=ptmp[:], in0=pred[:], in1=recip[:],
                                        op=OP.mult)
                yrow = ppool.tile([1, b_q], f32, tag="yrow")
                nc.vector.tensor_scalar(yrow[:], ptmp[:], svec[:, 0:1], None,
                                        OP.add)
                nc.sync.dma_start(out=y_d[:, :].rearrange("a b -> b a"), in_=yrow[:])

    return nc


def _get_nc(b_q=BQ, s_l=SL):
    key = (b_q, s_l)
    if key not in _CACHE:
        nc = _build(b_q, s_l)
        nc.finalize()
        _CACHE[key] = nc
    return _CACHE[key]


LAST_RESULTS = None


def kernel(x, support_set, params, _trace=False):
    global LAST_RESULTS
    from concourse.bass_utils import run_bass_kernel_spmd

    import ml_dtypes
    x = np.asarray(x, np.float32)
    support_set = np.asarray(support_set, np.float32)
    folded = _fold(params)
    xT = np.asarray(x.T, np.float32).astype(ml_dtypes.bfloat16)
    sT = np.asarray(support_set.T, np.float32).astype(ml_dtypes.bfloat16)

    nc = _get_nc()

    common = dict(
        w1=folded["w1"],
        w2=folded["w2"], w3=folded["w3"],
        wa1=folded["wa1"], wa2=folded["wa2"],
        wp1=folded["wp1"], wp2=folded["wp2"],
        vecs=folded["vecs"], bp1r=folded["bp1r"], svec=folded["svec"],
    )
    in_maps = []
    for r in range(NCORES):
        in_maps.append(dict(
            xqT=np.ascontiguousarray(xT[:, r * BQ:(r + 1) * BQ]),
            xsT=np.ascontiguousarray(sT[:, r * SL:(r + 1) * SL]),
            **common,
        ))

    kw = {}
    if _trace:
        kw = dict(trace=True, trace_cores=list(range(NCORES)))
    res = run_bass_kernel_spmd(nc, in_maps, list(range(NCORES)), **kw)
    LAST_RESULTS = res
    y = np.concatenate([res.results[r]["y"] for r in range(NCORES)], axis=0)
    return y.astype(np.float32)
